# revision 7
# baseline (speedup 1.0000x reference)
"""DiffLlama-style differential-attention block on 8 Trainium2 NeuronCores.

Sharding: data-parallel over batch (B=8 -> 1 batch element per core).
Per-core kernel (S=1024, H=1024, 8 heads x 128), all matmul inputs bf16
(fp32 PSUM accumulation, fp32 softmax/LayerNorm statistics):
  phase A/B: Q^T, K^T = (x Wq + bq)^T / sqrt(hd), (x Wk + bk)^T
  phase C:   V = x Wv, fused into VC_g = V_g - lam*V_{g+4} + bvc (+ ones col)
  phase D:   per head: scores^T = K_h^T Q_h (keys on partitions, 2 half-sets
             of PSUM), exp on ACT (2 instrs), AV matmul with ones-column ->
             unnormalized out + row-sum Z in one PSUM, normalize by 1/Z.
             Software-pipelined: QK(i); AV(i-1); exp(i) so PE never waits ACT.
  tail(c):   LayerNorm over hidden dim (bn_stats/bn_aggr + DVE pow(-0.5)),
             PE-transpose of Y (bf16), o_proj, +bo, DMA out.  LN emitted one
             pipeline step before the PE part.
"""

import numpy as np

B, S, H = 8, 1024, 1024
HEADS, HD = 8, 128
P = 128
KT = H // P   # 8 contraction tiles
ST = S // P   # 8 sequence tiles
HT = H // P   # 8 hidden tiles
LAMBDA_INIT = 0.8
LN_EPS = 1e-5
QCH = 256     # query-chunk for attention
JH = ST // 2  # j-tiles per scores half-set
NCH = 512     # free-dim chunk for projection matmuls
INV_SQRT_HD = 1.0 / np.sqrt(np.float32(HD))


def _bcast_ap(bass, v, parts=P):
    """[N] DRAM vector -> [parts, N] partition-broadcast AP for DMA."""
    return bass.AP(tensor=v.tensor, offset=v.offset, ap=[[0, parts], *v.ap])


def _build_nc(lam: float):
    import concourse.bacc as bacc
    import concourse.bass as bass
    import concourse.mybir as mybir
    import concourse.tile as tile
    from concourse.masks import make_identity

    f32 = mybir.dt.float32
    bf16 = mybir.dt.bfloat16
    AF = mybir.ActivationFunctionType
    OP = mybir.AluOpType

    nc = bacc.Bacc("TRN2", target_bir_lowering=False, debug=False, num_devices=8)

    xT = nc.dram_tensor("xT", [H, S], bf16, kind="ExternalInput").ap()
    wq = nc.dram_tensor("Wq", [H, H], bf16, kind="ExternalInput").ap()
    wk = nc.dram_tensor("Wk", [H, H], bf16, kind="ExternalInput").ap()
    wv = nc.dram_tensor("Wv", [H, H], bf16, kind="ExternalInput").ap()
    wo = nc.dram_tensor("Wo", [H, H], bf16, kind="ExternalInput").ap()
    bq = nc.dram_tensor("bq", [H], f32, kind="ExternalInput").ap()   # pre-scaled
    bk = nc.dram_tensor("bk", [H], f32, kind="ExternalInput").ap()
    bvc = nc.dram_tensor("bvc", [4 * P], f32, kind="ExternalInput").ap()
    out = nc.dram_tensor("out", [S, H], f32, kind="ExternalOutput").ap()

    with tile.TileContext(nc) as tc:
        with (
            tc.tile_pool(name="consts", bufs=1) as consts,
            tc.tile_pool(name="arena", bufs=2) as arena,       # xT / AO share slots
            tc.tile_pool(name="wpool", bufs=3) as wpool,       # streamed weights
            tc.tile_pool(name="qk", bufs=1) as qkpool,
            tc.tile_pool(name="vc", bufs=1) as vcpool,
            tc.tile_pool(name="expp", bufs=4) as expp,
            tc.tile_pool(name="ytp", bufs=2) as ytp,
            tc.tile_pool(name="outp", bufs=2) as outp,
            tc.tile_pool(name="small", bufs=3) as small,
            tc.tile_pool(name="ybfp", bufs=6) as ybfp,
        ):
            # ---- non-DMA constants ----
            ident = consts.tile([P, P], bf16)
            make_identity(nc, ident)
            i32 = mybir.dt.int32
            c_one = consts.tile([P, 1], i32)
            nc.vector.memset(c_one, 1)
            c_magic = consts.tile([P, 1], i32)
            nc.vector.memset(c_magic, 0x5F3759DF)
            # prewarm the exp table set on ScalarE while DMAs stream in, so
            # the first attention exp doesn't pay the ~2.7us ACT_TABLE_LOAD
            dummy = consts.tile([P, 1], f32)
            nc.scalar.activation(out=dummy, in_=c_one.bitcast(f32), func=AF.Exp)
            # prewarm the GpSimd tensor_scalar ucode library (first use pays
            # a library-load latency); LN normalize runs there mid-attention
            dummy2 = consts.tile([P, 1], f32)
            nc.gpsimd.tensor_scalar(
                out=dummy2, in0=dummy, scalar1=0.0, scalar2=None, op0=OP.mult,
            )

            # ---- input / weight loads ----
            # Everything streams on the SyncE HWDGE queue (ScalarE carries
            # ZERO dma triggers: each trigger costs ~0.6-0.8us of engine
            # time and starves the ACT drains / exp).  Few big triggers:
            # trigger-issue rate, not ring bandwidth, paced the old startup.
            xT_sb = arena.tile([P, KT, S], bf16, tag="big")
            xT_r = xT.rearrange("(kt p) s -> p kt s", p=P)

            def load_w(w):
                t = wpool.tile([P, KT, H], bf16, tag="w")
                r = w.rearrange("(kt p) n -> p kt n", p=P)
                nc.sync.dma_start(t, r)
                return t

            wq_sb = wpool.tile([P, KT, H], bf16, tag="w")
            wq_r = wq.rearrange("(kt p) n -> p kt n", p=P)
            nc.sync.dma_start(xT_sb[:, 0:4, 0:NCH], xT_r[:, 0:4, 0:NCH])
            nc.sync.dma_start(wq_sb[:, 0:4, 0:NCH], wq_r[:, 0:4, 0:NCH])
            bqT = consts.tile([P, HT], f32)
            nc.sync.dma_start(bqT, bq.rearrange("(ht p) -> p ht", p=P))
            bkT = consts.tile([P, HT], f32)
            nc.sync.dma_start(bkT, bk.rearrange("(ht p) -> p ht", p=P))
            nc.sync.dma_start(xT_sb[:, 4:KT, 0:NCH], xT_r[:, 4:KT, 0:NCH])
            nc.sync.dma_start(wq_sb[:, 4:KT, 0:NCH], wq_r[:, 4:KT, 0:NCH])
            nc.sync.dma_start(wq_sb[:, :, NCH:H], wq_r[:, :, NCH:H])
            nc.sync.dma_start(xT_sb[:, :, NCH:S], xT_r[:, :, NCH:S])
            wv_sb = load_w(wv)
            wk_sb = load_w(wk)

            QT_sb = qkpool.tile([P, HT, S], bf16, tag="qt")
            KT_sb = qkpool.tile([P, HT, S], bf16, tag="kt")

            def emit_proj(w_sb, bT, dst, scale2):
                for sc in range(S // NCH):
                    for ht in range(HT):
                        pq = ps_big.tile([P, NCH], f32, tag="pbig")
                        for kt in range(KT):
                            nc.tensor.matmul(
                                pq,
                                lhsT=w_sb[:, kt, ht * P:(ht + 1) * P],
                                rhs=xT_sb[:, kt, sc * NCH:(sc + 1) * NCH],
                                start=(kt == 0),
                                stop=(kt == KT - 1),
                            )
                        # bias+scale on ScalarE (idle during projections):
                        # out = Identity(psum * scale + bias)
                        nc.scalar.activation(
                            out=dst[:, ht, sc * NCH:(sc + 1) * NCH],
                            in_=pq,
                            func=AF.Identity,
                            bias=bT[:, ht:ht + 1],
                            scale=scale2,
                        )

            # ---- phases A-C under a deep projection PSUM pool (all 8
            # banks are free before attention starts) ----
            ps_proj = tc.tile_pool(name="ps_proj", bufs=4, space="PSUM")
            ps_big = ps_proj.__enter__()

            # ---- phase A: Q^T ----
            emit_proj(wq_sb, bqT, QT_sb, float(INV_SQRT_HD))

            # ---- phase B: V -> VC (before K^T so the last VC's DVE chain
            # hides under the K^T matmuls, which don't depend on it) ----
            bvc_bc = consts.tile([P, 4 * P], f32)
            nc.sync.dma_start(bvc_bc, _bcast_ap(bass, bvc))
            VC = vcpool.tile([P, ST, 4, HD + 1], bf16, tag="vc")
            nc.vector.memset(VC[:, :, :, HD:HD + 1], 1.0)
            for st in range(ST):
                p1 = ps_big.tile([P, NCH], f32, tag="pbig")
                p2 = ps_big.tile([P, NCH], f32, tag="pbig")
                for kt in range(KT):
                    nc.tensor.matmul(
                        p1,
                        lhsT=xT_sb[:, kt, st * P:(st + 1) * P],
                        rhs=wv_sb[:, kt, 0:NCH],
                        start=(kt == 0),
                        stop=(kt == KT - 1),
                    )
                for kt in range(KT):
                    nc.tensor.matmul(
                        p2,
                        lhsT=xT_sb[:, kt, st * P:(st + 1) * P],
                        rhs=wv_sb[:, kt, NCH:2 * NCH],
                        start=(kt == 0),
                        stop=(kt == KT - 1),
                    )
                t = small.tile([P, 4 * P], f32, tag="vtmp")
                # t = -lam * V2
                nc.vector.tensor_scalar(
                    out=t, in0=p2, scalar1=float(-lam), scalar2=None, op0=OP.mult,
                )
                # t += V1
                nc.vector.tensor_tensor(t, t, p1, OP.add)
                # VC[st] = t + bvc  (bf16)
                nc.vector.tensor_tensor(
                    VC[:, st, :, 0:HD],
                    t.rearrange("p (g d) -> p g d", g=4),
                    bvc_bc.rearrange("p (g d) -> p g d", g=4),
                    OP.add,
                )

            # ---- phase C: K^T ----
            emit_proj(wk_sb, bkT, KT_sb, 1.0)

            ps_proj.__exit__(None, None, None)

            # ---- attention-scope PSUM pools: 2x scores half-sets (4
            # banks) + small (AV/transpose, 2) + o_proj (2) ----
            _ps_sc = tc.tile_pool(name="ps_sc", bufs=2, space="PSUM")
            ps_sc = _ps_sc.__enter__()
            _ps_sm = tc.tile_pool(name="ps_sm", bufs=2, space="PSUM")
            ps_sm = _ps_sm.__enter__()
            _ps_o = tc.tile_pool(name="ps_o", bufs=2, space="PSUM")
            ps_o = _ps_o.__enter__()

            # ---- phase D + tails: software-pipelined attention ----
            wo_sb = load_w(wo)  # prefetch during attention
            AO = arena.tile([P, ST, H], f32, tag="big")
            n_qch = S // QCH          # 4 query chunks
            it_per = QCH // P         # 2 i-tiles per chunk
            NSUB = H // 512

            def emit_qk(c, h):
                """scores^T for (head h, query chunk c) into two half-sets."""
                halves = []
                for half in range(2):
                    ps = ps_sc.tile([P, JH, QCH], f32, tag="sc")
                    for j in range(JH):
                        jt = half * JH + j
                        nc.tensor.matmul(
                            ps[:, j, :],
                            lhsT=KT_sb[:, h, jt * P:(jt + 1) * P],
                            rhs=QT_sb[:, h, c * QCH:(c + 1) * QCH],
                            start=True,
                            stop=True,
                        )
                    halves.append(ps)
                return halves

            def emit_exp(halves):
                ets = []
                for ps in halves:
                    et = expp.tile([P, JH, QCH], bf16, tag="exp")
                    nc.scalar.activation(et, ps, AF.Exp)
                    ets.append(et)
                return ets

            def emit_av(c, h, ets):
                g = h % 4
                for it in range(it_per):
                    pav = ps_sm.tile([P, HD + 1], f32, tag="psm")
                    for jt in range(ST):
                        et = ets[jt // JH]
                        j = jt % JH
                        nc.tensor.matmul(
                            pav,
                            lhsT=et[:, j, it * P:(it + 1) * P],
                            rhs=VC[:, jt, g, :],
                            start=(jt == 0),
                            stop=(jt == ST - 1),
                        )
                    rec = small.tile([P, 1], f32, tag="rec")
                    nc.vector.reciprocal(rec, pav[:, HD:HD + 1])
                    nc.vector.tensor_scalar_mul(
                        AO[:, c * it_per + it, h * P:(h + 1) * P],
                        pav[:, 0:HD],
                        rec,
                    )

            def emit_tail_ln_st(st):
                """LayerNorm one s-tile; returns Y as a bf16 tile."""
                if True:
                    stats = small.tile([P, NSUB, 6], f32, tag="stats")
                    for sg in range(NSUB):
                        nc.vector.bn_stats(
                            out=stats[:, sg, :],
                            in_=AO[:, st, sg * 512:(sg + 1) * 512],
                        )
                    mv = small.tile([P, 2], f32, tag="mv")
                    nc.vector.bn_aggr(out=mv, in_=stats)
                    # rstd = rsqrt(var + eps) on DVE (Quake init + 2 Newton
                    # steps) — keeps ScalarE's exp table resident.
                    ve = small.tile([P, 1], f32, tag="ve")
                    nc.vector.tensor_scalar(
                        out=ve, in0=mv[:, 1:2], scalar1=LN_EPS, scalar2=None,
                        op0=OP.add,
                    )
                    rstd = small.tile([P, 1], f32, tag="rstd")
                    nc.vector.tensor_tensor(
                        rstd.bitcast(i32), ve.bitcast(i32), c_one,
                        OP.arith_shift_right,
                    )
                    nc.vector.tensor_tensor(
                        rstd.bitcast(i32), c_magic, rstd.bitcast(i32), OP.subtract,
                    )
                    nwt = small.tile([P, 1], f32, tag="nwt")
                    for _ in range(2):
                        nc.vector.tensor_tensor(nwt, ve, rstd, OP.mult)
                        nc.vector.tensor_tensor(nwt, nwt, rstd, OP.mult)
                        nc.vector.tensor_scalar(
                            out=nwt, in0=nwt, scalar1=-0.5, scalar2=1.5,
                            op0=OP.mult, op1=OP.add,
                        )
                        nc.vector.tensor_tensor(rstd, rstd, nwt, OP.mult)
                    # normalize on GpSimd (SBUF->SBUF): keeps the DVE FIFO
                    # free so AV-psum drains aren't queued behind this 0.8us
                    # op (that ordering stalled the PE on psum-bank reuse)
                    ybf = ybfp.tile([P, H], bf16, tag="ybf")
                    nc.gpsimd.tensor_scalar(
                        out=ybf,
                        in0=AO[:, st, :],
                        scalar1=mv[:, 0:1],
                        scalar2=rstd,
                        op0=OP.subtract,
                        op1=OP.mult,
                    )
                    return ybf

            def emit_tail_pe_st(st, ybf):
                """transpose + o_proj + store for one s-tile.  All 8 blocks
                land in one PSUM bank (8*128 bf16 = 2KB), one copy out."""
                yt = ytp.tile([P, HT, P], bf16, tag="yt")
                pt = ps_sm.tile([P, HT, P], bf16, tag="psm")
                for ht in range(HT):
                    nc.tensor.transpose(
                        pt[:, ht, :], ybf[:, ht * P:(ht + 1) * P], ident,
                    )
                nc.vector.tensor_copy(yt, pt)
                for ch in range(H // NCH):
                    pb = ps_o.tile([P, NCH], f32, tag="po")
                    for ht in range(HT):
                        nc.tensor.matmul(
                            pb,
                            lhsT=yt[:, ht, :],
                            rhs=wo_sb[:, ht, ch * NCH:(ch + 1) * NCH],
                            start=(ht == 0),
                            stop=(ht == HT - 1),
                        )
                    # psum drain on ACT (~0.3us vs 0.7us on DVE, and off the
                    # DVE FIFO); +bo is folded in on the host instead.
                    ob = outp.tile([P, NCH], f32, tag="ob")
                    nc.scalar.copy(out=ob, in_=pb)
                    nc.sync.dma_start(
                        out[st * P:(st + 1) * P, ch * NCH:(ch + 1) * NCH], ob,
                    )

            # Tail work queue: one task per pipeline step.  Per finished
            # chunk c: LN(st0), LN(st1), PE(st0), PE(st1) — the PE part of a
            # tile runs 2 steps after its LN, with QK/AV work in between.
            steps = [(c, h) for c in range(n_qch) for h in range(HEADS)]
            prev = None      # (c, h, ets)
            tasks = []       # queue of ("ln"|"pe", fn)
            state = {}       # st -> ybf tile

            def make_ln(st):
                return ("ln", lambda: state.__setitem__(st, emit_tail_ln_st(st)))

            def make_pe(st):
                return ("pe", lambda: emit_tail_pe_st(st, state.pop(st)))

            RESERVE_PE = 2   # PE tail tasks held back to fill the final flush

            def pop_task():
                if not tasks:
                    return
                kind = tasks[0][0]
                n_pe = sum(1 for k, _ in tasks if k == "pe")
                if kind == "ln" or n_pe > RESERVE_PE:
                    tasks.pop(0)[1]()

            for c, h in steps:
                halves = emit_qk(c, h)
                if prev is not None:
                    emit_av(*prev)
                    pop_task()
                    if prev[1] == HEADS - 1:
                        cc = prev[0]
                        st0, st1 = cc * it_per, cc * it_per + 1
                        tasks += [make_ln(st0), make_ln(st1),
                                  make_pe(st0), make_pe(st1)]
                prev = (c, h, emit_exp(halves))
            emit_av(*prev)
            cc = prev[0]
            st0, st1 = cc * it_per, cc * it_per + 1
            # LNs of the last chunk first (DVE), held-back PE tasks fill the
            # PE while those run, then the last chunk's PE parts (still warm)
            lns = [make_ln(st0), make_ln(st1)]
            pes = [make_pe(st0), make_pe(st1)]
            held = [t for t in tasks if t[0] == "pe"]
            rest = [t for t in tasks if t[0] == "ln"]
            for _, fn in rest + lns[:1]:
                fn()
            for _, fn in held[:1] + lns[1:] + held[1:] + pes:
                fn()
            _ps_o.__exit__(None, None, None)
            _ps_sm.__exit__(None, None, None)
            _ps_sc.__exit__(None, None, None)

    nc.compile()
    return nc


def kernel(hidden_states, Wq, bq, Wk, bk, Wv, bv, Wo, bo,
           lambda_q1, lambda_k1, lambda_q2, lambda_k2, ln_w, ln_b,
           _trace=False):
    import ml_dtypes
    from concourse.bass_utils import run_bass_kernel_spmd

    hidden_states = np.asarray(hidden_states, dtype=np.float32)
    Wq, Wk, Wv, Wo = (np.asarray(w, dtype=np.float32) for w in (Wq, Wk, Wv, Wo))
    bq, bk, bv, bo = (np.asarray(b, dtype=np.float32) for b in (bq, bk, bv, bo))
    ln_w = np.asarray(ln_w, dtype=np.float32)
    ln_b = np.asarray(ln_b, dtype=np.float32)

    lam1 = np.exp(np.sum(np.asarray(lambda_q1, np.float32) * np.asarray(lambda_k1, np.float32), dtype=np.float32))
    lam2 = np.exp(np.sum(np.asarray(lambda_q2, np.float32) * np.asarray(lambda_k2, np.float32), dtype=np.float32))
    lam = float(lam1 - lam2 + LAMBDA_INIT)

    bvc = (bv[: H // 2] - np.float32(lam) * bv[H // 2:]).astype(np.float32)
    # fold the LayerNorm affine + 0.2 prescale into the output projection:
    # 0.2*(z*ln_w + ln_b) @ Wo + bo == z @ (0.2*ln_w[:,None]*Wo) + (bo + 0.2*ln_b@Wo)
    s = np.float32(1.0 - LAMBDA_INIT)
    Wo_eff = (s * ln_w[:, None] * Wo).astype(np.float32)
    bo_eff = (bo + s * (ln_b @ Wo)).astype(np.float32)
    bq_s = (bq * INV_SQRT_HD).astype(np.float32)  # folded into ACT bias+scale

    nc = _build_nc(lam)

    bfl = ml_dtypes.bfloat16
    shared = dict(Wq=Wq.astype(bfl), Wk=Wk.astype(bfl), Wv=Wv.astype(bfl),
                  Wo=Wo_eff.astype(bfl), bq=bq_s, bk=bk, bvc=bvc)
    in_maps = []
    for i in range(B):
        m = dict(shared)
        m["xT"] = np.ascontiguousarray(hidden_states[i].T).astype(bfl)
        in_maps.append(m)

    res = run_bass_kernel_spmd(nc, in_maps, core_ids=list(range(B)), trace=_trace)
    out = np.stack([r["out"] for r in res.results], axis=0)
    out += bo_eff  # o_proj bias applied on host (device drains raw psum)
    if _trace:
        kernel._last_results = res
    return out



# revision 11
# speedup vs baseline: 1.2416x; 1.2416x over previous
"""DiffLlama-style differential-attention block on 8 Trainium2 NeuronCores.

Sharding: data-parallel over batch (B=8 -> 1 batch element per core).
Per-core kernel (S=1024, H=1024, 8 heads x 128), all matmul inputs bf16
(fp32 PSUM accumulation, fp32 softmax/LayerNorm statistics):
  phase A/B: Q^T, K^T = (x Wq + bq)^T / sqrt(hd), (x Wk + bk)^T
  phase C:   V = x Wv, fused into VC_g = V_g - lam*V_{g+4} + bvc (+ ones col)
  phase D:   per head: scores^T = K_h^T Q_h (keys on partitions, 2 half-sets
             of PSUM), exp on ACT (2 instrs), AV matmul with ones-column ->
             unnormalized out + row-sum Z in one PSUM, normalize by 1/Z.
             Software-pipelined: QK(i); AV(i-1); exp(i) so PE never waits ACT.
  tail(c):   LayerNorm over hidden dim (bn_stats/bn_aggr + DVE pow(-0.5)),
             PE-transpose of Y (bf16), o_proj, +bo, DMA out.  LN emitted one
             pipeline step before the PE part.
"""

import numpy as np

B, S, H = 8, 1024, 1024
HEADS, HD = 8, 128
P = 128
KT = H // P   # 8 contraction tiles
ST = S // P   # 8 sequence tiles
HT = H // P   # 8 hidden tiles
LAMBDA_INIT = 0.8
LN_EPS = 1e-5
QCH = 256     # query-chunk for attention
JH = ST // 2  # j-tiles per scores half-set
NCH = 512     # free-dim chunk for projection matmuls
INV_SQRT_HD = 1.0 / np.sqrt(np.float32(HD))


def _bcast_ap(bass, v, parts=P):
    """[N] DRAM vector -> [parts, N] partition-broadcast AP for DMA."""
    return bass.AP(tensor=v.tensor, offset=v.offset, ap=[[0, parts], *v.ap])


def _build_nc(lam: float):
    import concourse.bacc as bacc
    import concourse.bass as bass
    import concourse.mybir as mybir
    import concourse.tile as tile
    from concourse.masks import make_identity

    f32 = mybir.dt.float32
    bf16 = mybir.dt.bfloat16
    AF = mybir.ActivationFunctionType
    OP = mybir.AluOpType

    nc = bacc.Bacc("TRN2", target_bir_lowering=False, debug=False, num_devices=8)

    xT = nc.dram_tensor("xT", [H, S], bf16, kind="ExternalInput").ap()
    wq = nc.dram_tensor("Wq", [H, H], bf16, kind="ExternalInput").ap()
    wk = nc.dram_tensor("Wk", [H, H], bf16, kind="ExternalInput").ap()
    wv = nc.dram_tensor("Wv", [H, H], bf16, kind="ExternalInput").ap()
    wo = nc.dram_tensor("Wo", [H, H], bf16, kind="ExternalInput").ap()
    bq = nc.dram_tensor("bq", [H], f32, kind="ExternalInput").ap()   # pre-scaled
    bk = nc.dram_tensor("bk", [H], f32, kind="ExternalInput").ap()
    bvc = nc.dram_tensor("bvc", [4 * P], f32, kind="ExternalInput").ap()
    out = nc.dram_tensor("out", [S, H], f32, kind="ExternalOutput").ap()

    with tile.TileContext(nc) as tc:
        with (
            tc.tile_pool(name="consts", bufs=1) as consts,
            tc.tile_pool(name="arena", bufs=2) as arena,       # xT / AO share slots
            tc.tile_pool(name="wpool", bufs=3) as wpool,       # streamed weights
            tc.tile_pool(name="qk", bufs=1) as qkpool,
            tc.tile_pool(name="vc", bufs=1) as vcpool,
            tc.tile_pool(name="expp", bufs=4) as expp,
            tc.tile_pool(name="ytp", bufs=2) as ytp,
            tc.tile_pool(name="outp", bufs=2) as outp,
            tc.tile_pool(name="small", bufs=3) as small,
            tc.tile_pool(name="ybfp", bufs=6) as ybfp,
        ):
            # ---- non-DMA constants ----
            ident = consts.tile([P, P], bf16)
            make_identity(nc, ident)
            i32 = mybir.dt.int32
            c_one = consts.tile([P, 1], i32)
            nc.vector.memset(c_one, 1)
            c_magic = consts.tile([P, 1], i32)
            nc.vector.memset(c_magic, 0x5F3759DF)
            # ---- input / weight loads ----
            # Two HWDGE rings in parallel: SyncE carries xT/wv/biases/stores,
            # ScalarE carries wq/wk (its triggers all complete before the
            # first ACT drain is needed, so ACT is never clogged).  Fine
            # granularity at the head (0.25MB triggers) so the first Q-proj
            # matmuls start ~9us; coarse 1-2MB triggers after that since
            # trigger-issue rate (~0.65us each) was pacing the old startup.
            xT_sb = arena.tile([P, KT, S], bf16, tag="big")
            xT_r = xT.rearrange("(kt p) s -> p kt s", p=P)

            wq_sb = wpool.tile([P, KT, H], bf16, tag="w")
            wq_r = wq.rearrange("(kt p) n -> p kt n", p=P)
            for g in range(4):
                nc.sync.dma_start(
                    xT_sb[:, 2 * g:2 * g + 2, 0:NCH],
                    xT_r[:, 2 * g:2 * g + 2, 0:NCH],
                )
                nc.scalar.dma_start(
                    wq_sb[:, 2 * g:2 * g + 2, 0:NCH],
                    wq_r[:, 2 * g:2 * g + 2, 0:NCH],
                )
            bqT = consts.tile([P, HT], f32)
            nc.sync.dma_start(bqT, bq.rearrange("(ht p) -> p ht", p=P))
            bkT = consts.tile([P, HT], f32)
            nc.sync.dma_start(bkT, bk.rearrange("(ht p) -> p ht", p=P))
            # prewarm the exp table set on ScalarE (first attention exp
            # would otherwise pay the ~2.7us ACT_TABLE_LOAD)
            dummy = consts.tile([P, 1], f32)
            nc.scalar.activation(out=dummy, in_=c_one.bitcast(f32), func=AF.Exp)
            nc.scalar.dma_start(wq_sb[:, :, NCH:H], wq_r[:, :, NCH:H])
            nc.sync.dma_start(xT_sb[:, :, NCH:S], xT_r[:, :, NCH:S])
            wv_sb = wpool.tile([P, KT, H], bf16, tag="w")
            nc.sync.dma_start(wv_sb, wv.rearrange("(kt p) n -> p kt n", p=P))
            wk_sb = wpool.tile([P, KT, H], bf16, tag="w")
            nc.scalar.dma_start(wk_sb, wk.rearrange("(kt p) n -> p kt n", p=P))

            QT_sb = qkpool.tile([P, HT, S], bf16, tag="qt")
            KT_sb = qkpool.tile([P, HT, S], bf16, tag="kt")

            def emit_proj(w_sb, bT, dst, scale2):
                for sc in range(S // NCH):
                    for ht in range(HT):
                        pq = ps_big.tile([P, NCH], f32, tag="pbig")
                        for kt in range(KT):
                            nc.tensor.matmul(
                                pq,
                                lhsT=w_sb[:, kt, ht * P:(ht + 1) * P],
                                rhs=xT_sb[:, kt, sc * NCH:(sc + 1) * NCH],
                                start=(kt == 0),
                                stop=(kt == KT - 1),
                            )
                        # bias+scale on ScalarE (idle during projections):
                        # out = Identity(psum * scale + bias)
                        nc.scalar.activation(
                            out=dst[:, ht, sc * NCH:(sc + 1) * NCH],
                            in_=pq,
                            func=AF.Identity,
                            bias=bT[:, ht:ht + 1],
                            scale=scale2,
                        )

            # ---- phases A-C under a deep projection PSUM pool (all 8
            # banks are free before attention starts) ----
            ps_proj = tc.tile_pool(name="ps_proj", bufs=4, space="PSUM")
            ps_big = ps_proj.__enter__()

            # ---- phase A: Q^T ----
            emit_proj(wq_sb, bqT, QT_sb, float(INV_SQRT_HD))

            # ---- phase B: V -> VC (before K^T so the last VC's DVE chain
            # hides under the K^T matmuls, which don't depend on it) ----
            bvc_bc = consts.tile([P, 4 * P], f32)
            nc.sync.dma_start(bvc_bc, _bcast_ap(bass, bvc))
            VC = vcpool.tile([P, ST, 4, HD + 1], bf16, tag="vc")
            nc.vector.memset(VC[:, :, :, HD:HD + 1], 1.0)
            for st in range(ST):
                p1 = ps_big.tile([P, NCH], f32, tag="pbig")
                p2 = ps_big.tile([P, NCH], f32, tag="pbig")
                for kt in range(KT):
                    nc.tensor.matmul(
                        p1,
                        lhsT=xT_sb[:, kt, st * P:(st + 1) * P],
                        rhs=wv_sb[:, kt, 0:NCH],
                        start=(kt == 0),
                        stop=(kt == KT - 1),
                    )
                for kt in range(KT):
                    nc.tensor.matmul(
                        p2,
                        lhsT=xT_sb[:, kt, st * P:(st + 1) * P],
                        rhs=wv_sb[:, kt, NCH:2 * NCH],
                        start=(kt == 0),
                        stop=(kt == KT - 1),
                    )
                t = small.tile([P, 4 * P], f32, tag="vtmp")
                # t = -lam * V2
                nc.vector.tensor_scalar(
                    out=t, in0=p2, scalar1=float(-lam), scalar2=None, op0=OP.mult,
                )
                # t += V1
                nc.vector.tensor_tensor(t, t, p1, OP.add)
                # VC[st] = t + bvc  (bf16)
                nc.vector.tensor_tensor(
                    VC[:, st, :, 0:HD],
                    t.rearrange("p (g d) -> p g d", g=4),
                    bvc_bc.rearrange("p (g d) -> p g d", g=4),
                    OP.add,
                )

            # ---- phase C: K^T ----
            # wo prefetch trigger issued here: ScalarE only has sparse
            # K-proj drains during phase C, so the ~1us trigger is free
            wo_sb = wpool.tile([P, KT, H], bf16, tag="w")
            nc.scalar.dma_start(wo_sb, wo.rearrange("(kt p) n -> p kt n", p=P))
            emit_proj(wk_sb, bkT, KT_sb, 1.0)

            ps_proj.__exit__(None, None, None)

            # ---- attention-scope PSUM pools: 2x scores half-sets (4
            # banks) + small (AV/transpose, 2) + o_proj (2) ----
            _ps_sc = tc.tile_pool(name="ps_sc", bufs=2, space="PSUM")
            ps_sc = _ps_sc.__enter__()
            _ps_sm = tc.tile_pool(name="ps_sm", bufs=2, space="PSUM")
            ps_sm = _ps_sm.__enter__()
            _ps_o = tc.tile_pool(name="ps_o", bufs=2, space="PSUM")
            ps_o = _ps_o.__enter__()

            # ---- phase D + tails: software-pipelined attention ----
            AO = arena.tile([P, ST, H], f32, tag="big")
            n_qch = S // QCH          # 4 query chunks
            it_per = QCH // P         # 2 i-tiles per chunk
            NSUB = H // 512

            def emit_qk(c, h):
                """scores^T for (head h, query chunk c) into two half-sets."""
                halves = []
                for half in range(2):
                    ps = ps_sc.tile([P, JH, QCH], f32, tag="sc")
                    for j in range(JH):
                        jt = half * JH + j
                        nc.tensor.matmul(
                            ps[:, j, :],
                            lhsT=KT_sb[:, h, jt * P:(jt + 1) * P],
                            rhs=QT_sb[:, h, c * QCH:(c + 1) * QCH],
                            start=True,
                            stop=True,
                        )
                    halves.append(ps)
                return halves

            def emit_exp(halves):
                ets = []
                for ps in halves:
                    et = expp.tile([P, JH, QCH], bf16, tag="exp")
                    nc.scalar.activation(et, ps, AF.Exp)
                    ets.append(et)
                return ets

            def emit_av(c, h, ets):
                g = h % 4
                for it in range(it_per):
                    pav = ps_sm.tile([P, HD + 1], f32, tag="psm")
                    for jt in range(ST):
                        et = ets[jt // JH]
                        j = jt % JH
                        nc.tensor.matmul(
                            pav,
                            lhsT=et[:, j, it * P:(it + 1) * P],
                            rhs=VC[:, jt, g, :],
                            start=(jt == 0),
                            stop=(jt == ST - 1),
                        )
                    rec = small.tile([P, 1], f32, tag="rec")
                    nc.vector.reciprocal(rec, pav[:, HD:HD + 1])
                    nc.vector.tensor_scalar_mul(
                        AO[:, c * it_per + it, h * P:(h + 1) * P],
                        pav[:, 0:HD],
                        rec,
                    )

            def emit_ln_stats(st):
                """bn_stats for one s-tile (DVE, ~0.9us)."""
                stats = small.tile([P, NSUB, 6], f32, tag="stats")
                for sg in range(NSUB):
                    nc.vector.bn_stats(
                        out=stats[:, sg, :],
                        in_=AO[:, st, sg * 512:(sg + 1) * 512],
                    )
                return stats

            def emit_ln_norm(st, stats):
                """aggr + rsqrt Newton (DVE, small) + normalize on ACT."""
                mv = small.tile([P, 2], f32, tag="mv")
                nc.vector.bn_aggr(out=mv, in_=stats)
                # rstd = rsqrt(var + eps) on DVE (Quake init + 2 Newton
                # steps) — keeps ScalarE's exp table resident.
                ve = small.tile([P, 1], f32, tag="ve")
                nc.vector.tensor_scalar(
                    out=ve, in0=mv[:, 1:2], scalar1=LN_EPS, scalar2=None,
                    op0=OP.add,
                )
                rstd = small.tile([P, 1], f32, tag="rstd")
                nc.vector.tensor_tensor(
                    rstd.bitcast(i32), ve.bitcast(i32), c_one,
                    OP.arith_shift_right,
                )
                nc.vector.tensor_tensor(
                    rstd.bitcast(i32), c_magic, rstd.bitcast(i32), OP.subtract,
                )
                nwt = small.tile([P, 1], f32, tag="nwt")
                for _ in range(2):
                    nc.vector.tensor_tensor(nwt, ve, rstd, OP.mult)
                    nc.vector.tensor_tensor(nwt, nwt, rstd, OP.mult)
                    nc.vector.tensor_scalar(
                        out=nwt, in0=nwt, scalar1=-0.5, scalar2=1.5,
                        op0=OP.mult, op1=OP.add,
                    )
                    nc.vector.tensor_tensor(rstd, rstd, nwt, OP.mult)
                # normalize on ACT: ybf = Identity(AO*rstd + (-mu*rstd)).
                # Keeps the 0.8us op off the DVE FIFO so AV-psum drains are
                # never queued behind it (that ordering stalled the PE).
                nmr = small.tile([P, 1], f32, tag="nmr")
                nc.vector.tensor_scalar(
                    out=nmr, in0=mv[:, 0:1], scalar1=rstd, scalar2=-1.0,
                    op0=OP.mult, op1=OP.mult,
                )
                ybf = ybfp.tile([P, H], bf16, tag="ybf")
                nc.scalar.activation(
                    out=ybf, in_=AO[:, st, :], func=AF.Identity,
                    bias=nmr, scale=rstd,
                )
                return ybf

            def emit_pe_t(st, ybf):
                """transpose one s-tile.  All 8 blocks land in one PSUM bank
                (8*128 bf16 = 2KB), one DVE copy out."""
                yt = ytp.tile([P, HT, P], bf16, tag="yt")
                pt = ps_sm.tile([P, HT, P], bf16, tag="psm")
                for ht in range(HT):
                    nc.tensor.transpose(
                        pt[:, ht, :], ybf[:, ht * P:(ht + 1) * P], ident,
                    )
                nc.vector.tensor_copy(yt, pt)
                return yt

            def emit_pe_o(st, yt):
                """o_proj + store for one s-tile."""
                for ch in range(H // NCH):
                    pb = ps_o.tile([P, NCH], f32, tag="po")
                    for ht in range(HT):
                        nc.tensor.matmul(
                            pb,
                            lhsT=yt[:, ht, :],
                            rhs=wo_sb[:, ht, ch * NCH:(ch + 1) * NCH],
                            start=(ht == 0),
                            stop=(ht == HT - 1),
                        )
                    # psum drain on ACT (~0.3us vs 0.7us on DVE, and off the
                    # DVE FIFO); +bo is folded in on the host instead.
                    ob = outp.tile([P, NCH], f32, tag="ob")
                    nc.scalar.copy(out=ob, in_=pb)
                    nc.sync.dma_start(
                        out[st * P:(st + 1) * P, ch * NCH:(ch + 1) * NCH], ob,
                    )

            # Tail work queue: one task per pipeline step, each <=1us on any
            # one engine so AV-psum drains never sit behind a long DVE op.
            # Per finished chunk: stats(st0), norm(st0), stats(st1),
            # norm(st1), pe_t(st0), pe_o(st0), pe_t(st1), pe_o(st1).
            steps = [(c, h) for c in range(n_qch) for h in range(HEADS)]
            prev = None      # (c, h, ets)
            tasks = []       # queue of ("ln"|"pe", fn)
            state = {}       # (kind, st) -> tile

            def make_stats(st):
                return ("ln", lambda: state.__setitem__(
                    ("s", st), emit_ln_stats(st)))

            def make_norm(st):
                return ("ln", lambda: state.__setitem__(
                    ("y", st), emit_ln_norm(st, state.pop(("s", st)))))

            def make_pe_t(st):
                return ("pe", lambda: state.__setitem__(
                    ("t", st), emit_pe_t(st, state.pop(("y", st)))))

            def make_pe_o(st):
                return ("pe", lambda: emit_pe_o(st, state.pop(("t", st))))

            def chunk_tasks(cc):
                st0, st1 = cc * it_per, cc * it_per + 1
                return [make_stats(st0), make_norm(st0),
                        make_stats(st1), make_norm(st1),
                        make_pe_t(st0), make_pe_o(st0),
                        make_pe_t(st1), make_pe_o(st1)]

            RESERVE_PE = 2   # PE tail tasks held back to fill the final flush

            def pop_task():
                if not tasks:
                    return
                kind = tasks[0][0]
                n_pe = sum(1 for k, _ in tasks if k == "pe")
                if kind == "ln" or n_pe > RESERVE_PE:
                    tasks.pop(0)[1]()

            for c, h in steps:
                halves = emit_qk(c, h)
                if prev is not None:
                    emit_av(*prev)
                    pop_task()
                    if prev[1] == HEADS - 1:
                        tasks += chunk_tasks(prev[0])
                prev = (c, h, emit_exp(halves))
            emit_av(*prev)
            tasks += chunk_tasks(prev[0])
            # Flush: interleave remaining LN-side tasks (DVE/ACT) with the
            # held-back + final PE tasks so the PE stays fed to the end.
            lns = [t for t in tasks if t[0] == "ln"]
            pes = [t for t in tasks if t[0] == "pe"]
            li = pi = 0
            while li < len(lns) or pi < len(pes):
                if li < len(lns):
                    lns[li][1]()
                    li += 1
                if pi < len(pes):
                    pes[pi][1]()
                    pi += 1
            _ps_o.__exit__(None, None, None)
            _ps_sm.__exit__(None, None, None)
            _ps_sc.__exit__(None, None, None)

    nc.compile()
    return nc


def kernel(hidden_states, Wq, bq, Wk, bk, Wv, bv, Wo, bo,
           lambda_q1, lambda_k1, lambda_q2, lambda_k2, ln_w, ln_b,
           _trace=False):
    import ml_dtypes
    from concourse.bass_utils import run_bass_kernel_spmd

    hidden_states = np.asarray(hidden_states, dtype=np.float32)
    Wq, Wk, Wv, Wo = (np.asarray(w, dtype=np.float32) for w in (Wq, Wk, Wv, Wo))
    bq, bk, bv, bo = (np.asarray(b, dtype=np.float32) for b in (bq, bk, bv, bo))
    ln_w = np.asarray(ln_w, dtype=np.float32)
    ln_b = np.asarray(ln_b, dtype=np.float32)

    lam1 = np.exp(np.sum(np.asarray(lambda_q1, np.float32) * np.asarray(lambda_k1, np.float32), dtype=np.float32))
    lam2 = np.exp(np.sum(np.asarray(lambda_q2, np.float32) * np.asarray(lambda_k2, np.float32), dtype=np.float32))
    lam = float(lam1 - lam2 + LAMBDA_INIT)

    bvc = (bv[: H // 2] - np.float32(lam) * bv[H // 2:]).astype(np.float32)
    # fold the LayerNorm affine + 0.2 prescale into the output projection:
    # 0.2*(z*ln_w + ln_b) @ Wo + bo == z @ (0.2*ln_w[:,None]*Wo) + (bo + 0.2*ln_b@Wo)
    s = np.float32(1.0 - LAMBDA_INIT)
    Wo_eff = (s * ln_w[:, None] * Wo).astype(np.float32)
    bo_eff = (bo + s * (ln_b @ Wo)).astype(np.float32)
    bq_s = (bq * INV_SQRT_HD).astype(np.float32)  # folded into ACT bias+scale

    nc = _build_nc(lam)

    bfl = ml_dtypes.bfloat16
    shared = dict(Wq=Wq.astype(bfl), Wk=Wk.astype(bfl), Wv=Wv.astype(bfl),
                  Wo=Wo_eff.astype(bfl), bq=bq_s, bk=bk, bvc=bvc)
    in_maps = []
    for i in range(B):
        m = dict(shared)
        m["xT"] = np.ascontiguousarray(hidden_states[i].T).astype(bfl)
        in_maps.append(m)

    res = run_bass_kernel_spmd(nc, in_maps, core_ids=list(range(B)), trace=_trace)
    out = np.stack([r["out"] for r in res.results], axis=0)
    out += bo_eff  # o_proj bias applied on host (device drains raw psum)
    if _trace:
        kernel._last_results = res
    return out

